# revision 1
# baseline (speedup 1.0000x reference)
"""Trainium2 Bass kernel for ExpKernelModule (Hawkes positive-likelihood intensities).

out[b,i] = sum_{j<i} alpha[u,v]*beta[u,v]*exp(clip(-beta[u,v]*(t_i-t_j), -20, 0))
with u=ct[b,i], v=ct[b,j], alpha=softplus(log_alpha), beta=softplus(log_beta).

Device algorithm (one batch per core, data-parallel over B=8):
the exp argument  log(a*b) - beta*(t_i - t_j)  is a rank-64 bilinear form over
the (receiver, trigger) type one-hots:

  arg[i,j] = W1[v,i]*oh[v,j] + W2[v,i]*(t_j*oh[v,j])     (sum over v)
  W1[v,i] = C1[u_i,v] - B[u_i,v]*t_i,  W2[v,i] = B[u_i,v],  oh[v,j] = 1[ct_j==v]

Per 128-row tile, matmuls produce the full exp-argument block in PSUM; ScalarE
applies Exp with a fused accum_out row-sum. Row tile r only needs columns
[0, 128*(r+1)); the diagonal 128x128 block gets a -1e4 additive strict-lower
mask (VectorE) before Exp.

PE dtype: float16. Each fp32 operand is split into a hi/lo fp16 pair (22
effective mantissa bits); per-operand errors scale with term magnitude, and
large-magnitude args are exactly the dead ones (exp ~ 0). Two accumulating
matmuls per chunk cover all needed hi/lo cross products:
  mm1 K=128: [W1h, W1l, W2h, W2l] x [oh, oh, th*oh, th*oh]
  mm2 K=64:  [W2h, W2l]           x [tl*oh, tl*oh]
(Measured on HW: each matmul costs ~(398+N)/2.4GHz warm — 1 cyc/col stream
plus ~166ns of non-overlapped issue/drain (the per-MM LDWEIGHTS blocks
fill-after-fill pipelining and walrus's LDW dedup is unusable) — identically
for bf16/fp16, and K is nearly free, so fp16 costs the same as bf16 and keeps
fp32-level accuracy. Splitting into narrower-K matmuls with row-group
tile_position packing overlaps streams but loses to the extra per-MM
overhead. fp32 is 4 cyc/col; fp32r is a 12-bit-mantissa mode.)
Measured end-to-end error vs the fp32 reference: ~7e-6 absmax-relative.
Host prep is O(L*D) index gathers only.
"""

import numpy as np

B_, L, D, P = 8, 2048, 32, 128
NT = L // P  # row tiles per batch
MASK_NEG = -1.0e4
MMW = 512  # moving-operand width per matmul (ISA limit for fp32 PSUM out)
MM_DTYPE = "float16"  # fp16 pairs: ~7e-6 err; "bfloat16" pairs: ~4e-4 err

_cached = {}


def _build_nc():
    import concourse.bass as bass  # noqa: F401
    import concourse.tile as tile
    from concourse import bacc, mybir

    f32 = mybir.dt.float32
    f16 = getattr(mybir.dt, MM_DTYPE)

    nc = bacc.Bacc("TRN2", target_bir_lowering=False, debug=False, enable_asserts=False, num_devices=8)
    wa_d = nc.dram_tensor("wa", (4 * D, L), f16, kind="ExternalInput").ap()
    ra_d = nc.dram_tensor("ra", (4 * D, L), f16, kind="ExternalInput").ap()
    wb_d = nc.dram_tensor("wb", (2 * D, L), f16, kind="ExternalInput").ap()
    rb_d = nc.dram_tensor("rb", (2 * D, L), f16, kind="ExternalInput").ap()
    m_d = nc.dram_tensor("m", (P, P), f32, kind="ExternalInput").ap()
    # out[p, r] = row-sum for global row i = 128*r + p; one contiguous DMA
    o_d = nc.dram_tensor("o", (P, NT), f32, kind="ExternalOutput").ap()

    with tile.TileContext(nc) as tc:
        with (
            tc.tile_pool(name="singles", bufs=1) as singles,
            tc.tile_pool(name="psum_v7", bufs=2, space="PSUM") as psum,
            tc.tile_pool(name="acc", bufs=4) as accp,
        ):
            # Interleave input DMAs in consumption order (512-col pieces),
            # spread across the two HWDGE queues (sync + scalar) for overlap.
            wa_sb = singles.tile([4 * D, L], f16)
            wb_sb = singles.tile([2 * D, L], f16)
            ra_sb = singles.tile([4 * D, L], f16)
            rb_sb = singles.tile([2 * D, L], f16)
            m_sb = singles.tile([P, P], f32)
            # mm1 operands (ra/wa) ship one piece ahead of mm2's (rb/wb):
            # a tile's mm2 matmuls always trail its mm1s, so rb/wb can lag.
            def piece(eng, sb, dram, c0):
                sl = slice(c0, c0 + 512)
                eng.dma_start(sb[:, sl], dram[:, sl])

            piece(nc.sync, ra_sb, ra_d, 0)
            piece(nc.scalar, wa_sb, wa_d, 0)
            piece(nc.sync, ra_sb, ra_d, 512)
            piece(nc.scalar, wa_sb, wa_d, 512)
            piece(nc.sync, rb_sb, rb_d, 0)
            piece(nc.scalar, wb_sb, wb_d, 0)
            nc.scalar.dma_start(m_sb[:, :], m_d[:, :])
            for c0 in (1024, 1536):
                piece(nc.sync, ra_sb, ra_d, c0)
                piece(nc.scalar, wa_sb, wa_d, c0)
            for c0 in (512, 1024, 1536):
                piece(nc.sync, rb_sb, rb_d, c0)
                piece(nc.scalar, wb_sb, wb_d, c0)

            bias0 = singles.tile([P, 1], f32)
            nc.vector.memset(bias0[:, :], 0.0)
            acc = accp.tile([P, NT], f32)
            for rt in range(NT):
                ncols = P * (rt + 1)
                pt = psum.tile([P, L], f32)
                wsl = slice(rt * P, (rt + 1) * P)
                # all mm1 chunks first, then all mm2 chunks: consecutive PE
                # matmuls hit different PSUM banks, so fill overlaps drain
                # (same-bank accumulate pairs back-to-back serialize the PE).
                for c0 in range(0, ncols, MMW):
                    w_len = min(MMW, ncols - c0)
                    csl = slice(c0, c0 + w_len)
                    nc.tensor.matmul(
                        pt[:, csl], wa_sb[:, wsl], ra_sb[:, csl],
                        start=True, stop=False,
                    )
                for c0 in range(0, ncols, MMW):
                    w_len = min(MMW, ncols - c0)
                    csl = slice(c0, c0 + w_len)
                    nc.tensor.matmul(
                        pt[:, csl], wb_sb[:, wsl], rb_sb[:, csl],
                        start=False, stop=True,
                    )
                # strict-lower mask on the diagonal 128x128 block
                nc.vector.tensor_add(
                    pt[:, ncols - P : ncols], pt[:, ncols - P : ncols], m_sb[:, :]
                )
                nc.scalar.activation(
                    pt[:, :ncols],
                    pt[:, :ncols],
                    mybir.ActivationFunctionType.Exp,
                    bias=bias0[:, :],
                    accum_out=acc[:, rt : rt + 1],
                )
            nc.sync.dma_start(o_d[:, :], acc[:, :])

    nc.compile()
    return nc


def _softplus(x):
    return np.log1p(np.exp(-np.abs(x))) + np.maximum(x, 0.0)


def _host_prep(time_points, event_types, log_alpha, log_beta):
    t = np.asarray(time_points).astype(np.float64)  # (B, L)
    u = np.asarray(event_types).astype(np.int64)  # (B, L)
    A = _softplus(np.asarray(log_alpha).astype(np.float64))
    Bt = _softplus(np.asarray(log_beta).astype(np.float64))
    C1 = np.log(A * Bt)  # (D, D)

    if MM_DTYPE == "float16":
        f16 = np.float16
    else:
        import ml_dtypes

        f16 = ml_dtypes.bfloat16
    W1 = np.transpose(C1[u], (0, 2, 1)) - np.transpose(Bt[u], (0, 2, 1)) * t[:, None, :]
    W2 = np.transpose(Bt[u], (0, 2, 1))  # (B, D, L)
    W1h = W1.astype(f16); W1l = (W1 - W1h.astype(np.float64)).astype(f16)
    W2h = W2.astype(f16); W2l = (W2 - W2h.astype(np.float64)).astype(f16)
    th = t.astype(f16); tl = (t - th.astype(np.float64)).astype(f16)
    oh = (u[:, None, :] == np.arange(D)[None, :, None])  # (B, D, L) bool

    WA = np.concatenate([W1h, W1l, W2h, W2l], axis=1)  # (B, 4D, L) f16
    RA = np.concatenate(
        [oh, oh,
         th.astype(np.float64)[:, None, :] * oh,
         th.astype(np.float64)[:, None, :] * oh], axis=1
    ).astype(f16)  # (B, 4D, L)
    WB = np.concatenate([W2h, W2l], axis=1)  # (B, 2D, L)
    tlo = tl.astype(np.float64)[:, None, :] * oh
    RB = np.concatenate([tlo, tlo], axis=1).astype(f16)  # (B, 2D, L)
    mask = np.triu(np.full((P, P), MASK_NEG, dtype=np.float32), k=0)
    return WA, RA, WB, RB, mask


def _run(inputs, trace=False):
    from concourse.bass_utils import run_bass_kernel_spmd

    WA, RA, WB, RB, mask = _host_prep(
        inputs["time_points"],
        inputs["event_types"],
        inputs["log_alpha"],
        inputs["log_beta"],
    )
    if "nc" not in _cached:
        _cached["nc"] = _build_nc()
    nc = _cached["nc"]

    in_maps = [
        {"wa": WA[b], "ra": RA[b], "wb": WB[b], "rb": RB[b], "m": mask}
        for b in range(B_)
    ]
    bres = run_bass_kernel_spmd(
        nc, in_maps, core_ids=list(range(B_)), trace=trace,
        trace_cores=[0] if trace else None,
    )
    # o is (P, NT) with out[i=128*r+p] = o[p, r]
    out = np.stack(
        [bres.results[b]["o"].reshape(P, NT).T.reshape(L) for b in range(B_)], axis=0
    )
    return out.astype(np.float32), bres


def kernel(**inputs) -> np.ndarray:
    out, _ = _run(inputs, trace=False)
    return out



# revision 3
# speedup vs baseline: 1.4020x; 1.4020x over previous
"""Trainium2 Bass kernel for ExpKernelModule (Hawkes positive-likelihood intensities).

out[b,i] = sum_{j<i} alpha[u,v]*beta[u,v]*exp(clip(-beta[u,v]*(t_i-t_j), -20, 0))
with u=ct[b,i], v=ct[b,j], alpha=softplus(log_alpha), beta=softplus(log_beta).
(The -20 clip floor adds a*b*e^-20 per distant pair; ignoring it costs ~1e-5
rel err, well under tolerance.)

Device algorithm (one batch per core, data-parallel over B=8):
the exp argument  log(a*b) - beta*(t_i - t_j)  is a bilinear form over the
(receiver, trigger) type one-hots. Key numerical trick: round beta to fp16
FIRST (Br = fp16(beta)); then the model is exactly exponential in Br and the
Br rounding error cancels between the -Br*t_i and +Br*t_j terms (residual
error ~ (Br-beta)*dt_live <= 2.4e-4*20 = 5e-3 on live pairs). This lets ONE
K=128 fp16 matmul per 512-col chunk produce the full exp argument:

  W  = [W1h, W1l, Br, Br]            (4D=128, L) stationary, fp16
  RA = [oh, oh, th*oh, tl*oh]        (4D=128, L) moving,     fp16
  arg[i,j] = W1(i)[u_j] + Br[u_i,u_j]*(th_j + tl_j)
  W1[v,i] = log(a*b)[u_i,v] - Br[u_i,v]*t_i   (hi/lo fp16 pair, 22 bits)
  th/tl = hi/lo fp16 split of t (22 bits)

Per 128-row tile, matmuls fill the [128, ncols] exp-argument block in PSUM
(one matmul per 512-col chunk, distinct PSUM regions so fills overlap
drains); ScalarE applies Exp with a fused accum_out row-sum; the diagonal
128x128 block gets a -1e4 additive strict-lower mask (VectorE) first.
Input DMA is issued from the sync+vector queues (NOT scalar: the scalar
sequencer is the activation bottleneck) in consumption order.
"""

import numpy as np

B_, L, D, P = 8, 2048, 32, 128
NT = L // P  # row tiles per batch
MASK_NEG = -1.0e4
MMW = 512  # moving-operand width per matmul (ISA limit for fp32 PSUM out)

_cached = {}


def _build_nc():
    import concourse.bass as bass  # noqa: F401
    import concourse.tile as tile
    from concourse import bacc, mybir

    f32 = mybir.dt.float32
    f16 = mybir.dt.float16

    nc = bacc.Bacc("TRN2", target_bir_lowering=False, debug=False, enable_asserts=False, num_devices=8)
    wa_d = nc.dram_tensor("wa", (4 * D, L), f16, kind="ExternalInput").ap()
    ra_d = nc.dram_tensor("ra", (4 * D, L), f16, kind="ExternalInput").ap()
    m_d = nc.dram_tensor("m", (P, P), f32, kind="ExternalInput").ap()
    # out[p, r] = row-sum for global row i = 128*r + p; one contiguous DMA
    o_d = nc.dram_tensor("o", (P, NT), f32, kind="ExternalOutput").ap()

    with tile.TileContext(nc) as tc:
        with (
            tc.tile_pool(name="singles", bufs=1) as singles,
            tc.tile_pool(name="psum_v7", bufs=2, space="PSUM") as psum,
            tc.tile_pool(name="acc", bufs=4) as accp,
        ):
            wa_sb = singles.tile([4 * D, L], f16)
            ra_sb = singles.tile([4 * D, L], f16)
            m_sb = singles.tile([P, P], f32)

            # Interleave input DMAs in consumption order (1024-col pieces,
            # 2KB/partition rows), wa and ra racing on separate queues.
            def piece(eng, sb, dram, c0, w):
                sl = slice(c0, c0 + w)
                eng.dma_start(sb[:, sl], dram[:, sl])

            # first 512 cols ship alone so the first matmul starts earliest
            piece(nc.sync, ra_sb, ra_d, 0, 512)
            piece(nc.gpsimd, wa_sb, wa_d, 0, 512)
            nc.scalar.dma_start(m_sb[:, :], m_d[:, :])
            piece(nc.sync, ra_sb, ra_d, 512, 512)
            piece(nc.gpsimd, wa_sb, wa_d, 512, 512)
            piece(nc.sync, ra_sb, ra_d, 1024, 1024)
            piece(nc.gpsimd, wa_sb, wa_d, 1024, 1024)

            bias0 = singles.tile([P, 1], f32)
            nc.vector.memset(bias0[:, :], 0.0)
            acc = accp.tile([P, NT], f32)
            for rt in range(NT):
                ncols = P * (rt + 1)
                pt = psum.tile([P, L], f32)
                wsl = slice(rt * P, (rt + 1) * P)
                for c0 in range(0, ncols, MMW):
                    w_len = min(MMW, ncols - c0)
                    csl = slice(c0, c0 + w_len)
                    nc.tensor.matmul(
                        pt[:, csl], wa_sb[:, wsl], ra_sb[:, csl],
                        start=True, stop=True,
                    )
                # strict-lower mask on the diagonal 128x128 block
                nc.vector.tensor_add(
                    pt[:, ncols - P : ncols], pt[:, ncols - P : ncols], m_sb[:, :]
                )
                nc.scalar.activation(
                    pt[:, :ncols],
                    pt[:, :ncols],
                    mybir.ActivationFunctionType.Exp,
                    bias=bias0[:, :],
                    accum_out=acc[:, rt : rt + 1],
                )
            nc.sync.dma_start(o_d[:, :], acc[:, :])

    nc.compile()
    return nc


def _softplus(x):
    return np.log1p(np.exp(-np.abs(x))) + np.maximum(x, 0.0)


def _host_prep(time_points, event_types, log_alpha, log_beta):
    t = np.asarray(time_points).astype(np.float64)  # (B, L)
    u = np.asarray(event_types).astype(np.int64)  # (B, L)
    A = _softplus(np.asarray(log_alpha).astype(np.float64))
    Bt = _softplus(np.asarray(log_beta).astype(np.float64))
    C1 = np.log(A * Bt)  # (D, D)

    f16 = np.float16
    # round beta to fp16 FIRST; the model is then exactly exponential in Br
    Br = Bt.astype(f16).astype(np.float64)  # (D, D)

    Brg = np.transpose(Br[u], (0, 2, 1))  # (B, D, L): Br[u_i, v] at [b, v, i]
    W1 = np.transpose(C1[u], (0, 2, 1)) - Brg * t[:, None, :]
    W1h = W1.astype(f16)
    W1l = (W1 - W1h.astype(np.float64)).astype(f16)
    th = t.astype(f16)
    tl = (t - th.astype(np.float64)).astype(f16)
    oh = (u[:, None, :] == np.arange(D)[None, :, None])  # (B, D, L) bool

    WA = np.concatenate([W1h, W1l, Brg.astype(f16), Brg.astype(f16)], axis=1)
    RA = np.concatenate(
        [oh, oh,
         th.astype(np.float64)[:, None, :] * oh,
         tl.astype(np.float64)[:, None, :] * oh], axis=1
    ).astype(f16)  # (B, 4D, L)
    mask = np.triu(np.full((P, P), MASK_NEG, dtype=np.float32), k=0)
    return WA, RA, mask


def _run(inputs, trace=False):
    from concourse.bass_utils import run_bass_kernel_spmd

    WA, RA, mask = _host_prep(
        inputs["time_points"],
        inputs["event_types"],
        inputs["log_alpha"],
        inputs["log_beta"],
    )
    if "nc" not in _cached:
        _cached["nc"] = _build_nc()
    nc = _cached["nc"]

    in_maps = [{"wa": WA[b], "ra": RA[b], "m": mask} for b in range(B_)]
    bres = run_bass_kernel_spmd(
        nc, in_maps, core_ids=list(range(B_)), trace=trace,
        trace_cores=[0] if trace else None,
    )
    # o is (P, NT) with out[i=128*r+p] = o[p, r]
    out = np.stack(
        [bres.results[b]["o"].reshape(P, NT).T.reshape(L) for b in range(B_)], axis=0
    )
    return out.astype(np.float32), bres


def kernel(**inputs) -> np.ndarray:
    out, _ = _run(inputs, trace=False)
    return out


# revision 6
# speedup vs baseline: 1.4915x; 1.0638x over previous
"""Trainium2 Bass kernel for ExpKernelModule (Hawkes positive-likelihood intensities).

out[b,i] = sum_{j<i} a[u,v]*b[u,v]*exp(clip(-b[u,v]*(t_i-t_j), -20, 0)),
u=ct[b,i], v=ct[b,j], a=softplus(log_alpha), b=softplus(log_beta).
(Ignoring the -20 clip floor costs ~1e-5 rel err, well under tolerance.)

One batch per core (data-parallel over B=8). Key numerical trick used
throughout: round beta to fp16 FIRST (Br = fp16(beta)); every exponent is then
exactly linear in Br, so Br's rounding error cancels between the t_i and t_j
terms (residual ~ (Br-beta)*dt_live <= 2.4e-4*20 = 5e-3 on live pairs).

The L x L pairwise sum splits at 128-row-block granularity (tau_r := t[128r]):

NEAR (j in the same 128-block as i): exact pairwise via one K=128 fp16 matmul
per diagonal 128x128 block. The exp argument is a bilinear form over type
one-hots: W = [W1h, W1l, Br, Br] x RN = [oh, oh, t'h*oh, t'l*oh], with
W1[v,i] = log(ab)[u_i,v] - Br[u_i,v]*t'_i and t' = t - tau_blk re-referenced
per block (so hi/lo fp16 pairs carry ~22 bits on small-magnitude args).
A -1e4 strict-lower mask is added to the diagonal blocks (VectorE, broadcast
AP), then ScalarE Exp -> fp16 SBUF, then VectorE segmented row-sum reduce.

FAR (j in earlier blocks): per-block type aggregation collapses the O(L^2)
work to O(L*D). For block c define (all host-known args, device exps):
  Mhat_c[u,v] = sum_{j in c, u_j=v} ab[u,v] * e^{Br[u,v]*(t_j - tau_{c+1})}
computed as Exp(Garg) [128x32 per block] followed by a K=128 one-hot matmul.
A 15-step (32,32) rescale-scan on GpSimd builds the prefix
  F_r = s_r o F_{r-1} + Mhat_{r-1},   s_r = e^{Br*(tau_{r-1}-tau_r)}  (host)
so F_r[u,v] = sum_{j<128r, u_j=v} ab * e^{Br*(t_j-tau_r)}. A one-hot gather
matmul Y_r[i,:] = F_r[u_i,:] and X = Exp(-Br[u_i,v]*(t_i-tau_r)) give
  far[i] = sum_v X[i,v]*Y[i,v]   (VectorE multiply + segmented reduce).

Engine budget per core: PE ~46 small matmuls (~9us), ScalarE ~1.5k exp
columns (~4us, vs 17.4k cols if done fully pairwise), VectorE ~6us,
GpSimd scan ~4us chain, DMA ~1.7MB. All pipelined behind a ~10us fixed
framework/DMA startup.
"""

import numpy as np

B_, L, D, P = 8, 2048, 32, 128
NT = L // P  # 128-row blocks per batch
MASK_NEG = -1.0e4

_cached = {}


def _build_nc():
    import concourse.bass as bass  # noqa: F401
    import concourse.tile as tile
    from concourse import bacc, mybir

    f32 = mybir.dt.float32
    f16 = mybir.dt.float16
    Exp = mybir.ActivationFunctionType.Exp
    add = mybir.AluOpType.add

    nc = bacc.Bacc("TRN2", target_bir_lowering=False, debug=False, enable_asserts=False, num_devices=8)
    wn_d = nc.dram_tensor("wn", (4 * D, L), f16, kind="ExternalInput").ap()
    rn_d = nc.dram_tensor("rn", (4 * D, L), f16, kind="ExternalInput").ap()
    garg_d = nc.dram_tensor("garg", (P, NT * D), f32, kind="ExternalInput").ap()
    xarg_d = nc.dram_tensor("xarg", (P, NT * D), f32, kind="ExternalInput").ap()
    oh3_d = nc.dram_tensor("oh3", (P, NT * D), f16, kind="ExternalInput").ap()
    oht_d = nc.dram_tensor("oht", (D, L), f16, kind="ExternalInput").ap()
    s3_d = nc.dram_tensor("s3", (D, NT * D), f16, kind="ExternalInput").ap()
    m_d = nc.dram_tensor("m", (P, P), f16, kind="ExternalInput").ap()
    # out[p, r] = intensity for global row i = 128*r + p
    o_d = nc.dram_tensor("o", (P, NT), f32, kind="ExternalOutput").ap()

    with tile.TileContext(nc) as tc:
        with (
            tc.tile_pool(name="singles", bufs=1) as singles,
            tc.tile_pool(name="ps", bufs=1, space="PSUM") as psp,
        ):
            wn_sb = singles.tile([4 * D, L], f16)
            rn_sb = singles.tile([4 * D, L], f16)
            garg_sb = singles.tile([P, NT * D], f32)
            xarg_sb = singles.tile([P, NT * D], f32)
            oh3_sb = singles.tile([P, NT * D], f16)
            oht_sb = singles.tile([D, L], f16)
            s3_sb = singles.tile([D, NT * D], f16)
            m_sb = singles.tile([P, P], f16)
            ee_sb = singles.tile([P, NT * D], f16)
            xh_sb = singles.tile([P, NT * D], f16)
            nexp_sb = singles.tile([P, L], f16)
            t2_sb = singles.tile([P, NT * D], f16)
            f_sb = singles.tile([D, NT * D], f16)
            mhs_sb = singles.tile([D, NT * D], f16)
            nred = singles.tile([P, NT], f32)
            fred = singles.tile([P, NT], f32)
            acc = singles.tile([P, NT], f32)
            bias0 = singles.tile([P, 1], f32)

            near_ps = psp.tile([P, L], f32)
            mh_ps = psp.tile([D, NT * D], f32)
            y_ps = psp.tile([P, NT * D], f32)

            # ---- input DMA, consumption order, three queues ----
            nc.sync.dma_start(oh3_sb[:, :], oh3_d[:, :])
            nc.sync.dma_start(garg_sb[:, :], garg_d[:, :])
            nc.scalar.dma_start(s3_sb[:, :], s3_d[:, :])
            nc.scalar.dma_start(m_sb[:, :], m_d[:, :])
            nc.scalar.dma_start(xarg_sb[:, :], xarg_d[:, :])
            nc.scalar.dma_start(oht_sb[:, :], oht_d[:, :])
            for c0, w in ((0, 512), (512, 512), (1024, 1024)):
                nc.gpsimd.dma_start(wn_sb[:, c0 : c0 + w], wn_d[:, c0 : c0 + w])
                nc.sync.dma_start(rn_sb[:, c0 : c0 + w], rn_d[:, c0 : c0 + w])

            nc.vector.memset(bias0[:, :], 0.0)
            nc.vector.memset(y_ps[:, 0:D], 0.0)
            nc.vector.memset(xh_sb[:, 0:D], 0.0)

            # ---- ScalarE: the two tiny far-field exp batches ----
            nc.scalar.activation(
                ee_sb[:, : 15 * D], garg_sb[:, : 15 * D], Exp, bias=bias0[:, :]
            )
            nc.scalar.activation(
                xh_sb[:, D:], xarg_sb[:, D:], Exp, bias=bias0[:, :]
            )

            # ---- PE: Mhat matmuls (c = 0..14) ----
            for c in range(NT - 1):
                sl = slice(c * D, (c + 1) * D)
                nc.tensor.matmul(
                    mh_ps[:, sl], ee_sb[:, sl], oh3_sb[:, sl], start=True, stop=True
                )

            # ---- stage Mhat into SBUF (GpSimd cannot read PSUM) ----
            for b0, b1 in ((0, 4), (4, 8), (8, 12), (12, 15)):
                nc.vector.tensor_copy(
                    mhs_sb[:, b0 * D : b1 * D], mh_ps[:, b0 * D : b1 * D]
                )

            # ---- GpSimd: rescale-scan  F_r = s_r o F_{r-1} + Mhat_{r-1} ----
            nc.gpsimd.tensor_copy(f_sb[:, D : 2 * D], mhs_sb[:, 0:D])
            for r in range(2, NT):
                sl = slice(r * D, (r + 1) * D)
                slp = slice((r - 1) * D, r * D)
                nc.gpsimd.tensor_mul(f_sb[:, sl], f_sb[:, slp], s3_sb[:, sl])
                nc.gpsimd.tensor_add(f_sb[:, sl], f_sb[:, sl], mhs_sb[:, slp])

            # ---- PE near-field diagonal blocks + Y gathers, interleaved ----
            def near_mm(r):
                sl = slice(r * P, (r + 1) * P)
                nc.tensor.matmul(
                    near_ps[:, sl], wn_sb[:, sl], rn_sb[:, sl], start=True, stop=True
                )

            def y_mm(r):
                sl = slice(r * D, (r + 1) * D)
                nc.tensor.matmul(
                    y_ps[:, sl], oht_sb[:, r * P : (r + 1) * P], f_sb[:, sl],
                    start=True, stop=True,
                )

            ybatch = {0: range(1, 5), 1: range(5, 9), 2: range(9, 13), 3: range(13, NT)}
            for g in range(4):
                for r in range(4 * g, 4 * g + 4):
                    near_mm(r)
                for r in ybatch[g]:
                    y_mm(r)

            # ---- VectorE masks / ScalarE exps / VectorE reduces, per group ----
            mbc = m_sb[:, :].unsqueeze(1).broadcast_to([P, 4, P])
            for g in range(4):
                gsl = slice(g * 512, (g + 1) * 512)
                g3 = near_ps[:, gsl].rearrange("p (a b) -> p a b", b=P)
                nc.vector.tensor_add(g3, g3, mbc)
                nc.scalar.activation(
                    nexp_sb[:, gsl], near_ps[:, gsl], Exp, bias=bias0[:, :]
                )
                nc.vector.tensor_reduce(
                    nred[:, 4 * g : 4 * g + 4],
                    nexp_sb[:, gsl].rearrange("p (a b) -> p a b", b=P),
                    mybir.AxisListType.X, add,
                )

            # ---- far contraction + combine ----
            nc.vector.tensor_mul(t2_sb[:, :], xh_sb[:, :], y_ps[:, :])
            nc.vector.tensor_reduce(
                fred[:, :], t2_sb[:, :].rearrange("p (a b) -> p a b", b=D),
                mybir.AxisListType.X, add,
            )
            nc.vector.tensor_add(acc[:, :], nred[:, :], fred[:, :])
            nc.sync.dma_start(o_d[:, :], acc[:, :])

    nc.compile()
    return nc


def _softplus(x):
    return np.log1p(np.exp(-np.abs(x))) + np.maximum(x, 0.0)


def _host_prep(time_points, event_types, log_alpha, log_beta):
    t = np.asarray(time_points).astype(np.float64)  # (B, L)
    u = np.asarray(event_types).astype(np.int64)  # (B, L)
    A = _softplus(np.asarray(log_alpha).astype(np.float64))
    Bt = _softplus(np.asarray(log_beta).astype(np.float64))
    Br = Bt.astype(np.float16).astype(np.float64)  # fp16-rounded beta
    C1 = np.log(A * Bt)  # (D, D), true log(a*b)

    f16 = np.float16
    vD = np.arange(D)
    blk = np.arange(L) // P
    maps = []
    for b in range(t.shape[0]):
        ub, tb = u[b], t[b]
        tau = tb[::P].copy()  # (NT,)

        # near field: per-block re-referenced bilinear operands
        tp = tb - tau[blk]
        C1g = C1[ub].T  # (D, L): C1[u_i, v] at [v, i]
        Brg = Br[ub].T
        W1 = C1g - Brg * tp[None, :]
        W1h = W1.astype(f16)
        W1l = (W1 - W1h.astype(np.float64)).astype(f16)
        tph = tp.astype(f16)
        tpl = (tp - tph.astype(np.float64)).astype(f16)
        ohT = vD[:, None] == ub[None, :]  # (D, L)
        WN = np.concatenate([W1h, W1l, Brg.astype(f16), Brg.astype(f16)], 0)
        RN = np.concatenate(
            [ohT, ohT,
             tph.astype(np.float64)[None] * ohT,
             tpl.astype(np.float64)[None] * ohT], 0).astype(f16)

        # far field args (host-known), device does the exps
        ncut = (NT - 1) * P
        Garg = np.full((L, D), -100.0)
        taunext = tau[blk[:ncut] + 1]
        Garg[:ncut] = (Br[:, ub[:ncut]].T * (tb[:ncut] - taunext)[:, None]
                       + C1[:, ub[:ncut]].T)
        Xarg = -(Br[ub] * (tb - tau[blk])[:, None])
        Xarg[:P] = 0.0
        oh = (ub[:, None] == vD[None, :])

        def to3(a, dt):  # (L, D) -> (P, NT*D) with [p, c*D+v]
            return np.ascontiguousarray(
                a.reshape(NT, P, D).transpose(1, 0, 2).reshape(P, NT * D)
            ).astype(dt)

        s3 = np.zeros((D, NT, D))
        for r in range(2, NT):
            s3[:, r, :] = np.exp(Br * (tau[r - 1] - tau[r]))

        maps.append({
            "wn": WN, "rn": RN,
            "garg": to3(Garg, np.float32), "xarg": to3(Xarg, np.float32),
            "oh3": to3(oh, f16), "oht": ohT.astype(f16),
            "s3": s3.reshape(D, NT * D).astype(f16),
            "m": np.triu(np.full((P, P), MASK_NEG), k=0).astype(f16),
        })
    return maps


def _run(inputs, trace=False):
    from concourse.bass_utils import run_bass_kernel_spmd

    in_maps = _host_prep(
        inputs["time_points"],
        inputs["event_types"],
        inputs["log_alpha"],
        inputs["log_beta"],
    )
    if "nc" not in _cached:
        _cached["nc"] = _build_nc()
    nc = _cached["nc"]

    bres = run_bass_kernel_spmd(
        nc, in_maps, core_ids=list(range(B_)), trace=trace,
        trace_cores=[0] if trace else None,
    )
    # o is (P, NT) with out[i=128*r+p] = o[p, r]
    out = np.stack(
        [bres.results[b]["o"].reshape(P, NT).T.reshape(L) for b in range(B_)], axis=0
    )
    return out.astype(np.float32), bres


def kernel(**inputs) -> np.ndarray:
    out, _ = _run(inputs, trace=False)
    return out


# revision 13
# speedup vs baseline: 1.7207x; 1.1537x over previous
"""Trainium2 Bass kernel for ExpKernelModule (Hawkes positive-likelihood intensities).

out[b,i] = sum_{j<i} a[u,v]*b[u,v]*exp(clip(-b[u,v]*(t_i-t_j), -20, 0)),
u=ct[b,i], v=ct[b,j], a=softplus(log_alpha), b=softplus(log_beta).
(Ignoring the -20 clip floor costs ~1e-5 rel err, well under tolerance.)

One batch per core (data-parallel over B=8). Key numerical trick used
throughout: round beta to fp16 FIRST (Br = fp16(beta)); every exponent is then
exactly linear in Br, so Br's rounding error cancels between the t_i and t_j
terms (residual ~ (Br-beta)*dt_live <= 2.4e-4*20 = 5e-3 on live pairs).

The L x L pairwise sum splits at 128-row-block granularity (tau_r := t[128r]):

NEAR (j in the same 128-block as i): exact pairwise via one K=128 fp16 matmul
per diagonal 128x128 block. The exp argument is a bilinear form over type
one-hots: W = [W1h, W1l, Br, Br] x RN = [oh, oh, t'h*oh, t'l*oh], with
W1[v,i] = log(ab)[u_i,v] - Br[u_i,v]*t'_i and t' = t - tau_blk re-referenced
per block (hi/lo fp16 pairs carry ~22 bits on small-magnitude args).
A -1e4 strict-lower mask is added to the diagonal blocks (VectorE, broadcast
AP), then ScalarE Exp -> fp16 SBUF, then segmented row-sum reduces.

FAR (j in earlier blocks): per-block type aggregation collapses the O(L^2)
work to O(L*D). For block c define (args host-known, exps on device):
  Mhat_c[u,v] = sum_{j in c, u_j=v} ab[u,v] * e^{Br[u,v]*(t_j - tau_{c+1})}
computed as Exp(Garg) [128x32 per block] + a K=128 one-hot matmul that writes
PSUM in v-major layout (slot r=c+1, stride NT). The 15-step prefix recurrence
  F_r = s_r o F_{r-1} + Mhat_{r-1},   s_r = e^{Br*(tau_{r-1}-tau_r)}  (host)
collapses into ONE VectorE tensor_tensor_scan over [32, v*NT+r] (fp32 state;
s[*, r=0] = 0 resets the running state at each v boundary). Then a one-hot
gather matmul Y_r[i,:] = F_r[u_i,:] (strided rhs) and X = Exp(Xarg) give
  far[i] = sum_v X[i,v]*Y[i,v]   (multiply + segmented reduce).

Engine budget per core: PE 46 small matmuls (~9us), ScalarE ~1.5k exp columns
(~4us, vs 17.4k pairwise), VectorE ~6us, GpSimd ~4us, DMA ~1.8MB over three
queues. All pipelined behind the ~8us fixed framework/DMA startup.
"""

import numpy as np

B_, L, D, P = 8, 2048, 32, 128
NT = L // P  # 128-row blocks per batch
MASK_NEG = -1.0e4

_cached = {}


def _build_nc():
    import concourse.bass as bass  # noqa: F401
    import concourse.tile as tile
    from concourse import bacc, mybir

    f32 = mybir.dt.float32
    f16 = mybir.dt.float16
    Exp = mybir.ActivationFunctionType.Exp
    add = mybir.AluOpType.add
    mult = mybir.AluOpType.mult

    nc = bacc.Bacc("TRN2", target_bir_lowering=False, debug=False, enable_asserts=False, num_devices=8)
    wn_d = nc.dram_tensor("wn", (4 * D, L), f16, kind="ExternalInput").ap()
    rn_d = nc.dram_tensor("rn", (4 * D, L), f16, kind="ExternalInput").ap()
    garg_d = nc.dram_tensor("garg", (P, NT * D), f32, kind="ExternalInput").ap()
    xarg_d = nc.dram_tensor("xarg", (P, NT * D), f32, kind="ExternalInput").ap()
    oh3_d = nc.dram_tensor("oh3", (P, NT * D), f16, kind="ExternalInput").ap()
    oht_d = nc.dram_tensor("oht", (D, L), f16, kind="ExternalInput").ap()
    s3_d = nc.dram_tensor("s3", (D, D * NT), f16, kind="ExternalInput").ap()
    m_d = nc.dram_tensor("m", (P, P), f16, kind="ExternalInput").ap()
    # out[p, r] = intensity for global row i = 128*r + p
    o_d = nc.dram_tensor("o", (P, NT), f32, kind="ExternalOutput").ap()

    with tile.TileContext(nc) as tc:
        with (
            tc.tile_pool(name="singles", bufs=1) as singles,
            tc.tile_pool(name="ps", bufs=1, space="PSUM") as psp,
        ):
            wn_sb = singles.tile([4 * D, L], f16)
            rn_sb = singles.tile([4 * D, L], f16)
            garg_sb = singles.tile([P, NT * D], f32)
            xarg_sb = singles.tile([P, NT * D], f32)
            oh3_sb = singles.tile([P, NT * D], f16)
            oht_sb = singles.tile([D, L], f16)
            s3_sb = singles.tile([D, D * NT], f16)
            m_sb = singles.tile([P, P], f16)
            ee_sb = singles.tile([P, NT * D], f16)
            xh_sb = singles.tile([P, NT * D], f16)
            nexp_sb = singles.tile([P, L], f16)
            t2_sb = singles.tile([P, NT * D], f16)
            f_sb = singles.tile([D, D * NT], f16)  # [u, v*NT+r]
            nred = singles.tile([P, NT], f32)
            fred = singles.tile([P, NT], f32)
            acc = singles.tile([P, NT], f32)
            bias0 = singles.tile([P, 1], f32)
            scr = singles.tile([P, 1], f32)

            near_ps = psp.tile([P, L], f32)
            mh_ps = psp.tile([D, D * NT], f32)  # [u, v*NT+r], slot r=c+1
            y_ps = psp.tile([P, NT * D], f32)

            # ---- input DMA, consumption order, three queues ----
            nc.sync.dma_start(garg_sb[:, :], garg_d[:, :])
            nc.sync.dma_start(rn_sb[:, 0:1024], rn_d[:, 0:1024])
            nc.sync.dma_start(rn_sb[:, 1024:2048], rn_d[:, 1024:2048])
            nc.gpsimd.dma_start(oh3_sb[:, :], oh3_d[:, :])
            nc.gpsimd.dma_start(wn_sb[:, 0:1024], wn_d[:, 0:1024])
            nc.gpsimd.dma_start(wn_sb[:, 1024:2048], wn_d[:, 1024:2048])
            nc.scalar.dma_start(m_sb[:, :], m_d[:, :])
            nc.scalar.dma_start(s3_sb[:, :], s3_d[:, :])
            nc.scalar.dma_start(xarg_sb[:, :], xarg_d[:, :])
            nc.scalar.dma_start(oht_sb[:, :], oht_d[:, :])

            nc.vector.memset(bias0[:, :], 0.0)
            nc.vector.memset(y_ps[:, 0:D], 0.0)
            nc.vector.memset(xh_sb[:, 0:D], 0.0)
            nc.vector.memset(mh_ps[:, 0 : D * NT : NT], 0.0)  # scan slot r=0

            # warm the Exp table off the critical path
            nc.scalar.activation(scr[:, :], bias0[:, :], Exp, bias=bias0[:, :])

            # ---- ScalarE: the two tiny far-field exp batches ----
            nc.scalar.activation(
                ee_sb[:, : (NT - 1) * D], garg_sb[:, : (NT - 1) * D], Exp,
                bias=bias0[:, :],
            )
            nc.scalar.activation(
                xh_sb[:, D:], xarg_sb[:, D:], Exp, bias=bias0[:, :]
            )

            # ---- PE: Mhat matmuls (c = 0..14) -> v-major PSUM slot r=c+1 ----
            for c in range(NT - 1):
                sl = slice(c * D, (c + 1) * D)
                nc.tensor.matmul(
                    mh_ps[:, c + 1 : D * NT : NT], ee_sb[:, sl], oh3_sb[:, sl],
                    start=True, stop=True,
                )

            # ---- VectorE: the entire prefix recurrence in one scan ----
            # state[u,v*NT+r] = s3[u,v*NT+r]*state_prev + mh[u,v*NT+r]
            nc.vector.tensor_tensor_scan(
                f_sb[:, :], s3_sb[:, :], mh_ps[:, :], 0.0, mult, add
            )

            # ---- PE near-field diagonal blocks + Y gathers, interleaved ----
            def near_mm(r):
                sl = slice(r * P, (r + 1) * P)
                nc.tensor.matmul(
                    near_ps[:, sl], wn_sb[:, sl], rn_sb[:, sl], start=True, stop=True
                )

            def y_mm(r):
                nc.tensor.matmul(
                    y_ps[:, r * D : (r + 1) * D],
                    oht_sb[:, r * P : (r + 1) * P],
                    f_sb[:, r : D * NT : NT],
                    start=True, stop=True,
                )

            ybatch = {0: range(1, 5), 1: range(5, 9), 2: range(9, 13), 3: range(13, NT)}
            for g in range(4):
                for r in range(4 * g, 4 * g + 4):
                    near_mm(r)
                for r in ybatch[g]:
                    y_mm(r)

            # ---- masks / exps (ScalarE) / segmented row-sums (VectorE) ----
            mbc = m_sb[:, :].unsqueeze(1).broadcast_to([P, 4, P])
            for g in range(4):
                gsl = slice(g * 512, (g + 1) * 512)
                g3 = near_ps[:, gsl].rearrange("p (a b) -> p a b", b=P)
                nc.vector.tensor_add(g3, g3, mbc)
                nc.scalar.activation(
                    nexp_sb[:, gsl], near_ps[:, gsl], Exp, bias=bias0[:, :]
                )
                nc.vector.tensor_reduce(
                    nred[:, 4 * g : 4 * g + 4],
                    nexp_sb[:, gsl].rearrange("p (a b) -> p a b", b=P),
                    mybir.AxisListType.X, add,
                )

            # ---- far contraction + combine ----
            nc.vector.tensor_mul(t2_sb[:, :], xh_sb[:, :], y_ps[:, :])
            nc.vector.tensor_reduce(
                fred[:, :], t2_sb[:, :].rearrange("p (a b) -> p a b", b=D),
                mybir.AxisListType.X, add,
            )
            nc.vector.tensor_add(acc[:, :], nred[:, :], fred[:, :])
            nc.sync.dma_start(o_d[:, :], acc[:, :])

    nc.compile()
    return nc


def _softplus(x):
    return np.log1p(np.exp(-np.abs(x))) + np.maximum(x, 0.0)


def _host_prep(time_points, event_types, log_alpha, log_beta):
    t = np.asarray(time_points).astype(np.float64)  # (B, L)
    u = np.asarray(event_types).astype(np.int64)  # (B, L)
    A = _softplus(np.asarray(log_alpha).astype(np.float64))
    Bt = _softplus(np.asarray(log_beta).astype(np.float64))
    Br = Bt.astype(np.float16).astype(np.float64)  # fp16-rounded beta
    C1 = np.log(A * Bt)  # (D, D), true log(a*b)

    f16 = np.float16
    vD = np.arange(D)
    blk = np.arange(L) // P
    maps = []
    for b in range(t.shape[0]):
        ub, tb = u[b], t[b]
        tau = tb[::P].copy()  # (NT,)

        # near field: per-block re-referenced bilinear operands
        tp = tb - tau[blk]
        C1g = C1[ub].T  # (D, L): C1[u_i, v] at [v, i]
        Brg = Br[ub].T
        W1 = C1g - Brg * tp[None, :]
        W1h = W1.astype(f16)
        W1l = (W1 - W1h.astype(np.float64)).astype(f16)
        tph = tp.astype(f16)
        tpl = (tp - tph.astype(np.float64)).astype(f16)
        ohT = vD[:, None] == ub[None, :]  # (D, L)
        WN = np.concatenate([W1h, W1l, Brg.astype(f16), Brg.astype(f16)], 0)
        RN = np.concatenate(
            [ohT, ohT,
             tph.astype(np.float64)[None] * ohT,
             tpl.astype(np.float64)[None] * ohT], 0).astype(f16)

        # far field args (host-known), device does the exps
        ncut = (NT - 1) * P
        Garg = np.full((L, D), -100.0)
        taunext = tau[blk[:ncut] + 1]
        Garg[:ncut] = (Br[:, ub[:ncut]].T * (tb[:ncut] - taunext)[:, None]
                       + C1[:, ub[:ncut]].T)
        Xarg = -(Br[ub] * (tb - tau[blk])[:, None])
        Xarg[:P] = 0.0
        oh = (ub[:, None] == vD[None, :])

        def to3(a, dt):  # (L, D) -> (P, NT*D) with [p, c*D+v]
            return np.ascontiguousarray(
                a.reshape(NT, P, D).transpose(1, 0, 2).reshape(P, NT * D)
            ).astype(dt)

        # s3[u, v*NT+r]: 0 for r<2 (state reset at each v boundary / F_1=Mhat_0)
        s3 = np.zeros((D, D, NT))
        for r in range(2, NT):
            s3[:, :, r] = np.exp(Br * (tau[r - 1] - tau[r]))

        maps.append({
            "wn": WN, "rn": RN,
            "garg": to3(Garg, np.float32), "xarg": to3(Xarg, np.float32),
            "oh3": to3(oh, f16), "oht": ohT.astype(f16),
            "s3": s3.reshape(D, D * NT).astype(f16),
            "m": np.triu(np.full((P, P), MASK_NEG), k=0).astype(f16),
        })
    return maps


def _run(inputs, trace=False):
    from concourse.bass_utils import run_bass_kernel_spmd

    in_maps = _host_prep(
        inputs["time_points"],
        inputs["event_types"],
        inputs["log_alpha"],
        inputs["log_beta"],
    )
    if "nc" not in _cached:
        _cached["nc"] = _build_nc()
    nc = _cached["nc"]

    bres = run_bass_kernel_spmd(
        nc, in_maps, core_ids=list(range(B_)), trace=trace,
        trace_cores=[0] if trace else None,
    )
    # o is (P, NT) with out[i=128*r+p] = o[p, r]
    out = np.stack(
        [bres.results[b]["o"].reshape(P, NT).T.reshape(L) for b in range(B_)], axis=0
    )
    return out.astype(np.float32), bres


def kernel(**inputs) -> np.ndarray:
    out, _ = _run(inputs, trace=False)
    return out


# revision 20
# speedup vs baseline: 1.8542x; 1.0776x over previous
"""Trainium2 Bass kernel for ExpKernelModule (Hawkes positive-likelihood intensities).

out[b,i] = sum_{j<i} a[u,v]*b[u,v]*exp(clip(-b[u,v]*(t_i-t_j), -20, 0)),
u=ct[b,i], v=ct[b,j], a=softplus(log_alpha), b=softplus(log_beta).
(Ignoring the -20 clip floor costs ~1e-5 rel err, well under tolerance.)

One batch per core (data-parallel over B=8). Key numerical trick used
throughout: round beta to fp16 FIRST (Br = fp16(beta)); every exponent is then
exactly linear in Br, so Br's rounding error cancels between the t_i and t_j
terms (residual ~ (Br-beta)*dt_live <= 2.4e-4*20 = 5e-3 on live pairs).

The L x L pairwise sum splits at 128-row-block granularity (tau_r := t[128r]):

NEAR (j in the same 128-block as i): exact pairwise via one K=128 fp16 matmul
per diagonal 128x128 block. The exp argument is a bilinear form over type
one-hots: W = [W1h, W1l, Br, Br] x RN = [oh, oh, t'h*oh, t'l*oh], with
W1[v,i] = log(ab)[u_i,v] - Br[u_i,v]*t'_i and t' = t - tau_blk re-referenced
per block (hi/lo fp16 pairs carry ~22 bits on small-magnitude args).
A -1e4 strict-lower mask is added to the diagonal blocks (VectorE, broadcast
AP), then ScalarE Exp -> fp16 SBUF, then segmented row-sum reduces.

FAR (j in earlier blocks): per-block type aggregation collapses the O(L^2)
work to O(L*D). For block c define (args host-known, exps on device):
  Mhat_c[u,v] = sum_{j in c, u_j=v} ab[u,v] * e^{Br[u,v]*(t_j - tau_{c+1})}
computed as Exp(Garg) [128x32 per block] + a K=128 one-hot matmul that writes
PSUM in v-major layout (slot r=c+1, stride NT). The 15-step prefix recurrence
  F_r = s_r o F_{r-1} + Mhat_{r-1},   s_r = e^{Br*(tau_{r-1}-tau_r)}  (host)
collapses into ONE VectorE tensor_tensor_scan over [32, v*NT+r] (fp32 state;
s[*, r=0] = 0 resets the running state at each v boundary). Then a one-hot
gather matmul Y_r[i,:] = F_r[u_i,:] (strided rhs) and X = Exp(Xarg) give
  far[i] = sum_v X[i,v]*Y[i,v]   (multiply + segmented reduce).

Engine budget per core: PE 46 small matmuls (~9us), ScalarE ~1.5k exp columns
(~4us, vs 17.4k pairwise), VectorE ~6us, GpSimd ~4us, DMA ~1.8MB over three
queues. All pipelined behind the ~8us fixed framework/DMA startup.
"""

import numpy as np

B_, L, D, P = 8, 2048, 32, 128
NT = L // P  # 128-row blocks per batch
MASK_NEG = -1.0e4

_cached = {}


def _build_nc():
    import concourse.bass as bass  # noqa: F401
    import concourse.tile as tile
    from concourse import bacc, mybir

    f32 = mybir.dt.float32
    f16 = mybir.dt.float16
    Exp = mybir.ActivationFunctionType.Exp
    add = mybir.AluOpType.add
    mult = mybir.AluOpType.mult

    nc = bacc.Bacc("TRN2", target_bir_lowering=False, debug=False, enable_asserts=False, num_devices=8)
    wn_d = nc.dram_tensor("wn", (4 * D, L), f16, kind="ExternalInput").ap()
    rn_d = nc.dram_tensor("rn", (4 * D, L), f16, kind="ExternalInput").ap()
    garg_d = nc.dram_tensor("garg", (P, NT * D), f32, kind="ExternalInput").ap()
    xarg_d = nc.dram_tensor("xarg", (P, NT * D), f32, kind="ExternalInput").ap()
    oh3_d = nc.dram_tensor("oh3", (P, NT * D), f16, kind="ExternalInput").ap()
    oht_d = nc.dram_tensor("oht", (D, L), f16, kind="ExternalInput").ap()
    s3_d = nc.dram_tensor("s3", (D, D * NT), f16, kind="ExternalInput").ap()
    m_d = nc.dram_tensor("m", (P, P), f16, kind="ExternalInput").ap()
    # out[p, r] = intensity for global row i = 128*r + p
    o_d = nc.dram_tensor("o", (P, NT), f32, kind="ExternalOutput").ap()

    with tile.TileContext(nc) as tc:
        with (
            tc.tile_pool(name="singles", bufs=1) as singles,
            tc.tile_pool(name="ps", bufs=1, space="PSUM") as psp,
        ):
            wn_sb = singles.tile([4 * D, L], f16)
            rn_sb = singles.tile([4 * D, L], f16)
            garg_sb = singles.tile([P, NT * D], f32)
            xarg_sb = singles.tile([P, NT * D], f32)
            oh3_sb = singles.tile([P, NT * D], f16)
            oht_sb = singles.tile([D, L], f16)
            s3_sb = singles.tile([D, D * NT], f16)
            m_sb = singles.tile([P, P], f16)
            ee_sb = singles.tile([P, NT * D], f16)
            xh_sb = singles.tile([P, NT * D], f16)
            nexp_sb = singles.tile([P, L], f16)
            t2_sb = singles.tile([P, NT * D], f16)
            f_sb = singles.tile([D, D * NT], f16)  # [u, v*NT+r]
            nexp32_sb = singles.tile([P, L], f32)
            nred = singles.tile([P, NT], f32)
            fred = singles.tile([P, NT], f32)
            acc = singles.tile([P, NT], f32)
            bias0 = singles.tile([P, 1], f32)
            scr = singles.tile([P, 1], f32)

            near_ps = psp.tile([P, L], f32)
            mh_ps = psp.tile([D, D * NT], f32)  # [u, v*NT+r], slot r=c+1
            y_ps = psp.tile([P, NT * D], f32)

            # ---- input DMA, consumption order, three queues ----
            nc.sync.dma_start(garg_sb[:, :], garg_d[:, :])
            nc.sync.dma_start(oh3_sb[:, :], oh3_d[:, :])
            nc.sync.dma_start(rn_sb[:, 0:1024], rn_d[:, 0:1024])
            nc.gpsimd.dma_start(wn_sb[:, 0:1024], wn_d[:, 0:1024])
            nc.gpsimd.dma_start(wn_sb[:, 1024:2048], wn_d[:, 1024:2048])
            nc.scalar.dma_start(m_sb[:, :], m_d[:, :])
            nc.scalar.dma_start(s3_sb[:, :], s3_d[:, :])
            nc.scalar.dma_start(xarg_sb[:, :], xarg_d[:, :])
            nc.scalar.dma_start(rn_sb[:, 1024:2048], rn_d[:, 1024:2048])
            nc.scalar.dma_start(oht_sb[:, :], oht_d[:, :])

            nc.vector.memset(bias0[:, :], 0.0)
            nc.vector.memset(y_ps[:, 0:D], 0.0)
            nc.vector.memset(xh_sb[:, 0:D], 0.0)
            nc.vector.memset(mh_ps[:, 0 : D * NT : NT], 0.0)  # scan slot r=0

            # warm the Exp table off the critical path
            nc.scalar.activation(scr[:, :], bias0[:, :], Exp, bias=bias0[:, :])

            # ---- ScalarE: the two tiny far-field exp batches ----
            nc.scalar.activation(
                ee_sb[:, : (NT - 1) * D], garg_sb[:, : (NT - 1) * D], Exp,
                bias=bias0[:, :],
            )
            nc.scalar.activation(
                xh_sb[:, D:], xarg_sb[:, D:], Exp, bias=bias0[:, :]
            )

            # ---- PE: Mhat matmuls (c = 0..14) -> v-major PSUM slot r=c+1 ----
            for c in range(NT - 1):
                sl = slice(c * D, (c + 1) * D)
                nc.tensor.matmul(
                    mh_ps[:, c + 1 : D * NT : NT], ee_sb[:, sl], oh3_sb[:, sl],
                    start=True, stop=True,
                )

            # ---- VectorE: the entire prefix recurrence in one scan ----
            # state[u,v*NT+r] = s3[u,v*NT+r]*state_prev + mh[u,v*NT+r]
            nc.vector.tensor_tensor_scan(
                f_sb[:, :], s3_sb[:, :], mh_ps[:, :], 0.0, mult, add
            )

            # ---- PE near-field diagonal blocks + Y gathers, interleaved ----
            def near_mm(r):
                sl = slice(r * P, (r + 1) * P)
                nc.tensor.matmul(
                    near_ps[:, sl], wn_sb[:, sl], rn_sb[:, sl], start=True, stop=True
                )

            def y_mm(r):
                nc.tensor.matmul(
                    y_ps[:, r * D : (r + 1) * D],
                    oht_sb[:, r * P : (r + 1) * P],
                    f_sb[:, r : D * NT : NT],
                    start=True, stop=True,
                )

            ybatch = {0: range(1, 5), 1: range(5, 9), 2: range(9, 13), 3: range(13, NT)}
            for g in range(4):
                for r in range(4 * g, 4 * g + 4):
                    near_mm(r)
                for r in ybatch[g]:
                    y_mm(r)

            # ---- per group: exp (ScalarE, f32 out — unmasked stays finite),
            #      0/1 strict-lower mask multiply (GpSimd), row-sums (VectorE)
            mbc = m_sb[:, :].unsqueeze(1).broadcast_to([P, 4, P])
            for g in range(4):
                gsl = slice(g * 512, (g + 1) * 512)
                nc.scalar.activation(
                    nexp32_sb[:, gsl], near_ps[:, gsl], Exp, bias=bias0[:, :]
                )
                g3 = nexp_sb[:, gsl].rearrange("p (a b) -> p a b", b=P)
                nc.gpsimd.tensor_mul(
                    g3, nexp32_sb[:, gsl].rearrange("p (a b) -> p a b", b=P), mbc
                )
                nc.vector.tensor_reduce(
                    nred[:, 4 * g : 4 * g + 4], g3, mybir.AxisListType.X, add
                )

            # ---- far contraction + combine ----
            nc.vector.tensor_mul(t2_sb[:, :], xh_sb[:, :], y_ps[:, :])
            nc.vector.tensor_reduce(
                fred[:, :], t2_sb[:, :].rearrange("p (a b) -> p a b", b=D),
                mybir.AxisListType.X, add,
            )
            nc.vector.tensor_add(acc[:, :], nred[:, :], fred[:, :])
            nc.sync.dma_start(o_d[:, :], acc[:, :])

    nc.compile()
    return nc


def _softplus(x):
    return np.log1p(np.exp(-np.abs(x))) + np.maximum(x, 0.0)


def _host_prep(time_points, event_types, log_alpha, log_beta):
    t = np.asarray(time_points).astype(np.float64)  # (B, L)
    u = np.asarray(event_types).astype(np.int64)  # (B, L)
    A = _softplus(np.asarray(log_alpha).astype(np.float64))
    Bt = _softplus(np.asarray(log_beta).astype(np.float64))
    Br = Bt.astype(np.float16).astype(np.float64)  # fp16-rounded beta
    C1 = np.log(A * Bt)  # (D, D), true log(a*b)

    f16 = np.float16
    vD = np.arange(D)
    blk = np.arange(L) // P
    maps = []
    for b in range(t.shape[0]):
        ub, tb = u[b], t[b]
        tau = tb[::P].copy()  # (NT,)

        # near field: per-block re-referenced bilinear operands
        tp = tb - tau[blk]
        C1g = C1[ub].T  # (D, L): C1[u_i, v] at [v, i]
        Brg = Br[ub].T
        W1 = C1g - Brg * tp[None, :]
        W1h = W1.astype(f16)
        W1l = (W1 - W1h.astype(np.float64)).astype(f16)
        tph = tp.astype(f16)
        tpl = (tp - tph.astype(np.float64)).astype(f16)
        ohT = vD[:, None] == ub[None, :]  # (D, L)
        WN = np.concatenate([W1h, W1l, Brg.astype(f16), Brg.astype(f16)], 0)
        RN = np.concatenate(
            [ohT, ohT,
             tph.astype(np.float64)[None] * ohT,
             tpl.astype(np.float64)[None] * ohT], 0).astype(f16)

        # far field args (host-known), device does the exps
        ncut = (NT - 1) * P
        Garg = np.full((L, D), -100.0)
        taunext = tau[blk[:ncut] + 1]
        Garg[:ncut] = (Br[:, ub[:ncut]].T * (tb[:ncut] - taunext)[:, None]
                       + C1[:, ub[:ncut]].T)
        Xarg = -(Br[ub] * (tb - tau[blk])[:, None])
        Xarg[:P] = 0.0
        oh = (ub[:, None] == vD[None, :])

        def to3(a, dt):  # (L, D) -> (P, NT*D) with [p, c*D+v]
            return np.ascontiguousarray(
                a.reshape(NT, P, D).transpose(1, 0, 2).reshape(P, NT * D)
            ).astype(dt)

        # s3[u, v*NT+r]: 0 for r<2 (state reset at each v boundary / F_1=Mhat_0)
        s3 = np.zeros((D, D, NT))
        for r in range(2, NT):
            s3[:, :, r] = np.exp(Br * (tau[r - 1] - tau[r]))

        maps.append({
            "wn": WN, "rn": RN,
            "garg": to3(Garg, np.float32), "xarg": to3(Xarg, np.float32),
            "oh3": to3(oh, f16), "oht": ohT.astype(f16),
            "s3": s3.reshape(D, D * NT).astype(f16),
            "m": np.tril(np.ones((P, P)), k=-1).astype(f16),
        })
    return maps


def _run(inputs, trace=False):
    from concourse.bass_utils import run_bass_kernel_spmd

    in_maps = _host_prep(
        inputs["time_points"],
        inputs["event_types"],
        inputs["log_alpha"],
        inputs["log_beta"],
    )
    if "nc" not in _cached:
        _cached["nc"] = _build_nc()
    nc = _cached["nc"]

    bres = run_bass_kernel_spmd(
        nc, in_maps, core_ids=list(range(B_)), trace=trace,
        trace_cores=[0] if trace else None,
    )
    # o is (P, NT) with out[i=128*r+p] = o[p, r]
    out = np.stack(
        [bres.results[b]["o"].reshape(P, NT).T.reshape(L) for b in range(B_)], axis=0
    )
    return out.astype(np.float32), bres


def kernel(**inputs) -> np.ndarray:
    out, _ = _run(inputs, trace=False)
    return out
